# revision 10
# baseline (speedup 1.0000x reference)
"""DGCN (GCNConv + self/change terms) on 8 Trainium2 NeuronCores.

Design (dst-sharded graph parallelism, cost-model-tuned to 91.4us/core):
  - 392 dst tiles of 128 nodes; each core owns 49, dealt from a (lo, hi)
    descending sort with a pairwise same-core swap hill-climb so per-slot
    max-over-cores edge counts are tight (~2% padding).
  - Within each (group, half) gather region, slots pack back-to-back at
    max-profile lengths; only regions are block-aligned. Blocks spanning a
    slot boundary get one matmul+one-hot per covered slot (per-view dl
    columns). Groups [12,12,12,12,1]; the tiny last group shortens the
    post-gather drain.
  - Edges sorted by dst within each (tile, half), so one-hots span narrow
    dst ranges (mean width ~32); the first view per PSUM chain is
    full-width with start=True to initialize all columns.
  - Gathers: bf16 rows of dinv-prescaled x, 1024-idx calls (measured
    balance point of Pool descriptor-gen vs transfer pacing) on 4 SWDGE
    queues; int16 idx limit forces the lo/hi table split at row 32768.
  - Algebra: out = z@C + x@B2 + dinv^2*x@C + b' with C = Wc(I+Wt),
    B2 = W0-Wt, b' = bc(I+Wt); self-loops and bias ride the flipped
    epilogue (o[dout, dst]) whose per-partition bias rides the ACT copy.
  - Queue placement: first ix slice at SP head; dl/dd tables via ACT;
    mid-kernel group writes via SP; final write via ACT (no sem hop).
    Output is written transposed and un-permuted on host.
"""

import numpy as np

N_NODES = 50000
D = 128
N_CORES = 8
TILE = 128
BLK = 128
HALF = 32768
MAX_IDX_CALL = 1024  # per dma_gather call
N_SWDGE_QUEUES = 4
OFFLOAD_K = 0        # one-hot gpsimd offload disabled (head-of-line blocks gathers)

N_TILES = 392
TPC = N_TILES // N_CORES          # 49
GROUP_SIZES = [12, 12, 12, 12, 1]  # slots per group; tiny last = short tail
N_PAD = N_TILES * TILE
ROWS_PC = TPC * TILE

_NC_CACHE = {}
_LAST_RESULTS = None


def _host_prep(x, edge_index, Wc, bc, W0, Wt):
    import ml_dtypes

    bf16 = ml_dtypes.bfloat16
    n, d = x.shape
    src = np.asarray(edge_index[0], dtype=np.int64)
    dst = np.asarray(edge_index[1], dtype=np.int64)

    deg = (np.bincount(dst, minlength=N_PAD) + 1).astype(np.float32)
    dinv = (1.0 / np.sqrt(deg)).astype(np.float32)

    src_a = src
    dst_a = dst

    tile_g = dst_a // TILE
    ishi = (src_a >= HALF).astype(np.int64)
    order0 = np.lexsort((dst_a, ishi, tile_g))
    src_s = src_a[order0]
    dst_s = dst_a[order0]
    tile_s = tile_g[order0]

    counts = np.bincount(tile_s, minlength=N_TILES)
    starts = np.zeros(N_TILES + 1, np.int64)
    starts[1:] = np.cumsum(counts)
    lo_counts = np.bincount(tile_g[ishi == 0], minlength=N_TILES)
    hi_counts = counts - lo_counts

    # octet balancing: sort tiles by (lo, hi) DESCENDING and deal octet i
    # across the 8 cores as slot i; biggest slots first so the final (tiny)
    # group holds the smallest tiles. Then a pairwise same-core swap
    # hill-climb tightens sum(max lo + max hi) over octets.
    order_t = np.lexsort((hi_counts, lo_counts))[::-1]
    assign = order_t.reshape(TPC, N_CORES).copy()   # [slot, core] -> tile
    for _sweep in range(3):
        improved = 0
        alo = lo_counts[assign]
        ahi = hi_counts[assign]
        for i in range(TPC):
            for j in range(i + 1, TPC):
                base_cost = (alo[i].max() + ahi[i].max()
                             + alo[j].max() + ahi[j].max())
                for k in range(N_CORES):
                    alo[i, k], alo[j, k] = alo[j, k], alo[i, k]
                    ahi[i, k], ahi[j, k] = ahi[j, k], ahi[i, k]
                    new_cost = (alo[i].max() + ahi[i].max()
                                + alo[j].max() + ahi[j].max())
                    if new_cost < base_cost:
                        assign[i, k], assign[j, k] = assign[j, k], assign[i, k]
                        base_cost = new_cost
                        improved += 1
                    else:
                        alo[i, k], alo[j, k] = alo[j, k], alo[i, k]
                        ahi[i, k], ahi[j, k] = ahi[j, k], ahi[i, k]
        if improved == 0:
            break
    L_lo = lo_counts[assign].max(axis=1)            # [slot] max-profile lens
    L_hi = hi_counts[assign].max(axis=1)

    assert sum(GROUP_SIZES) == TPC
    grp_slots = []
    s0_ = 0
    for gs in GROUP_SIZES:
        grp_slots.append(list(range(s0_, s0_ + gs)))
        s0_ += gs

    # region/block/view layout
    NBASE = 0                    # running global block count
    callplan = []                # per group: (is_hi, local_b0, nb_blocks)
    grp_nb = []                  # blocks per group
    grp_off = []                 # global block offset per group
    slot_views_h = [([], []) for _ in range(TPC)]  # per half: (global_block, dl_col)
    # per-slot placement info for table building:
    place = {}                   # (slot, half) -> (region_pos, region_glb_b0)
    nview = 0
    for q, sl in enumerate(grp_slots):
        grp_off.append(NBASE)
        calls = []
        gb = 0                   # group-local block counter
        for h, L in ((0, L_lo), (1, L_hi)):
            pos = 0
            covered = []         # (slot, p0, p1) in region slot-positions
            for i in sl:
                li = int(L[i])
                place[(i, h)] = (pos, NBASE + gb)
                if li > 0:
                    covered.append((i, pos, pos + li))
                pos += li
            rblocks = -(-pos // BLK)
            # views: block b covers slot i iff ranges overlap
            for b in range(rblocks):
                lo_p, hi_p = b * BLK, (b + 1) * BLK
                for (i, p0, p1) in covered:
                    if p0 < hi_p and p1 > lo_p:
                        slot_views_h[i][h].append((NBASE + gb + b, nview))
                        nview += 1
            # calls (block-aligned, <=8 blocks each)
            b0 = gb
            nb = rblocks
            while nb > 0:
                take = min(nb, MAX_IDX_CALL // BLK)
                calls.append((h, b0, take))
                b0 += take
                nb -= take
            gb += rblocks
        grp_nb.append(gb)
        NBASE += gb
        callplan.append(calls)
    NB = NBASE
    NVIEWS = nview

    # tables
    idx_flat = np.zeros((N_CORES, NB * BLK), np.int32)
    dd_t = np.zeros((N_CORES, BLK, NB), np.float32)
    dl_t = np.full((N_CORES, BLK, NVIEWS), 1000.0, np.float32)
    for k in range(N_CORES):
        for i in range(TPC):
            g = assign[i, k]
            s0 = int(starts[g])
            clo, chi = int(lo_counts[g]), int(hi_counts[g])
            base = g * TILE
            for h, cnt, shift, pos0 in ((0, clo, 0, s0), (1, chi, HALF, s0 + clo)):
                if cnt == 0:
                    continue
                rpos, rgb0 = place[(i, h)]
                e0 = rgb0 * BLK + rpos
                idx_flat[k, e0:e0 + cnt] = src_s[pos0:pos0 + cnt] - shift
                # dd_t is [BLK, NB] with flat pos j -> [j%BLK, j//BLK]
                jj = np.arange(e0, e0 + cnt)
                dd_t[k][jj % BLK, jj // BLK] = dinv[dst_s[pos0:pos0 + cnt]]
        # dl per view (pass B below; geometry shared with pass A)

    def _view_rows(k):
        for i in range(TPC):
            g = assign[i, k]
            s0 = int(starts[g])
            clo, chi = int(lo_counts[g]), int(hi_counts[g])
            base = g * TILE
            for h in (0, 1):
                cnt = clo if h == 0 else chi
                pos0 = s0 if h == 0 else s0 + clo
                rpos, rgb0 = place.get((i, h), (None, None))
                if rpos is None:
                    continue
                li = int((L_lo if h == 0 else L_hi)[i])
                b_lo = rgb0 * BLK + rpos
                b_hi = b_lo + li
                for jj, (gb, vcol) in enumerate(slot_views_h[i][h]):
                    blk_lo, blk_hi = gb * BLK, (gb + 1) * BLK
                    if not (b_lo < blk_hi and b_hi > blk_lo):
                        continue
                    p_start = max(b_lo, blk_lo)
                    p_end = min(b_hi, blk_hi)
                    r0 = p_start - blk_lo
                    eo0 = p_start - b_lo
                    m = min(cnt - eo0, p_end - p_start)
                    yield (vcol, jj, r0, m, pos0 + eo0, base)

    d0_v = np.full(NVIEWS, TILE, np.int64)
    d1_v = np.zeros(NVIEWS, np.int64)
    first_v = np.zeros(NVIEWS, bool)
    for k in range(N_CORES):
        for (vcol, jj, r0, m, p0, base) in _view_rows(k):
            if jj == 0:
                first_v[vcol] = True
            if m > 0:
                dv = dst_s[p0:p0 + m] - base
                d0_v[vcol] = min(d0_v[vcol], int(dv.min()))
                d1_v[vcol] = max(d1_v[vcol], int(dv.max()) + 1)
    d0_v[first_v] = 0
    d1_v[first_v] = TILE
    bad = d1_v <= d0_v
    d0_v[bad] = 0
    d1_v[bad] = 1

    W16 = NB * (BLK // 16)
    for k in range(N_CORES):
        for (vcol, jj, r0, m, p0, base) in _view_rows(k):
            if m > 0:
                dl_t[k][r0:r0 + m, vcol] = (
                    dst_s[p0:p0 + m] - base - d0_v[vcol]).astype(np.float32)

    ix16 = np.empty((N_CORES, BLK, W16), np.int16)
    for k in range(N_CORES):
        v = idx_flat[k].astype(np.int16).reshape(W16, 16).T
        ix16[k] = np.tile(v, (8, 1))

    Wc64 = np.asarray(Wc, np.float64)
    Wt64 = np.asarray(Wt, np.float64)
    W064 = np.asarray(W0, np.float64)
    bc64 = np.asarray(bc, np.float64)
    B1 = np.eye(d) + Wt64
    C = (Wc64 @ B1).astype(np.float32)
    B2 = (W064 - Wt64).astype(np.float32)
    bp = (bc64 @ B1).astype(np.float32)

    x_pad = np.zeros((N_PAD, d), np.float32)
    x_pad[:n] = np.asarray(x, np.float32)
    xs = (x_pad * dinv[:, None]).astype(bf16)
    xself = x_pad * (dinv * dinv)[:, None]

    iota = np.broadcast_to(np.arange(TILE, dtype=np.float32),
                           (BLK, TILE)).astype(bf16)

    node_ids = np.empty((N_CORES, ROWS_PC), np.int64)
    for k in range(N_CORES):
        for i in range(TPC):
            g = assign[i, k]
            node_ids[k, i * TILE:(i + 1) * TILE] = np.arange(
                g * TILE, (g + 1) * TILE)

    in_maps = []
    for k in range(N_CORES):
        m = {
            "x_lo": xs[:HALF],
            "x_hi": xs[HALF:],
            "xT": np.ascontiguousarray(x_pad[node_ids[k]].T.astype(bf16)),
            "xsT": np.ascontiguousarray(xself[node_ids[k]].T.astype(bf16)),
            "ix16": ix16[k],
            "dl": dl_t[k].astype(bf16),
            "dd": dd_t[k].astype(bf16),
            "cw": C.astype(bf16),
            "b2w": B2.astype(bf16),
            "bpc": bp.reshape(d, 1),
            "iota": np.ascontiguousarray(iota),
        }
        in_maps.append(m)

    meta = dict(
        grp_slots=grp_slots, grp_nb=grp_nb, grp_off=grp_off,
        callplan=callplan, slot_views=slot_views_h, NB=NB, NVIEWS=NVIEWS,
        d0_v=d0_v, d1_v=d1_v,
        W16=W16, hi_rows=N_PAD - HALF, node_ids=node_ids,
    )
    return in_maps, meta


def _build_nc(meta, ablate=()):
    import concourse.bacc as bacc
    import concourse.mybir as mybir
    import concourse.tile as tile
    from concourse import library_config

    f32 = mybir.dt.float32
    bf16 = mybir.dt.bfloat16
    i16 = mybir.dt.int16
    eq, mul = mybir.AluOpType.is_equal, mybir.AluOpType.mult
    ident = mybir.ActivationFunctionType.Identity

    grp_slots, grp_nb, grp_off = meta["grp_slots"], meta["grp_nb"], meta["grp_off"]
    callplan, slot_views = meta["callplan"], meta["slot_views"]
    NB, NVIEWS, W16 = meta["NB"], meta["NVIEWS"], meta["W16"]
    d0_v, d1_v = meta["d0_v"], meta["d1_v"]

    nc = bacc.Bacc(
        "TRN2",
        target_bir_lowering=False,
        debug=False,
        num_devices=N_CORES,
        num_swdge_queues=N_SWDGE_QUEUES,
    )
    x_lo = nc.declare_dram_parameter("x_lo", [HALF, D], bf16, isOutput=False)
    x_hi = nc.declare_dram_parameter("x_hi", [meta["hi_rows"], D], bf16,
                                     isOutput=False)
    xT = nc.declare_dram_parameter("xT", [D, ROWS_PC], bf16, isOutput=False)
    xsT = nc.declare_dram_parameter("xsT", [D, ROWS_PC], bf16, isOutput=False)
    ix16 = nc.declare_dram_parameter("ix16", [BLK, W16], i16, isOutput=False)
    dl = nc.declare_dram_parameter("dl", [BLK, NVIEWS], bf16, isOutput=False)
    dd = nc.declare_dram_parameter("dd", [BLK, NB], bf16, isOutput=False)
    cw = nc.declare_dram_parameter("cw", [D, D], bf16, isOutput=False)
    b2w = nc.declare_dram_parameter("b2w", [D, D], bf16, isOutput=False)
    bpc = nc.declare_dram_parameter("bpc", [D, 1], f32, isOutput=False)
    iota = nc.declare_dram_parameter("iota", [BLK, TILE], bf16, isOutput=False)
    out = nc.declare_dram_parameter("out", [D, ROWS_PC], bf16, isOutput=True)

    with tile.TileContext(nc) as tc:
        with (
            tc.tile_pool(name="const", bufs=1) as cpool,
            tc.tile_pool(name="tbl", bufs=1) as tpool,
            tc.tile_pool(name="gather", bufs=2) as gpool,
            tc.tile_pool(name="oh", bufs=16) as ohpool,
            tc.tile_pool(name="z", bufs=24) as zsbpool,
            tc.tile_pool(name="og", bufs=2) as ogpool,
            tc.tile_pool(name="zps", bufs=6, space="PSUM") as zpool,
            tc.tile_pool(name="ops", bufs=2, space="PSUM") as opool,
        ):
            nc.gpsimd.load_library(library_config.mlp)
            ix_sb = tpool.tile([BLK, W16], i16)
            sl0 = min(16, int(grp_nb[0])) * 8
            nc.sync.dma_start(out=ix_sb[:, :sl0], in_=ix16[:, :sl0])
            c_sb = cpool.tile([D, D], bf16)
            nc.sync.dma_start(out=c_sb[:], in_=cw[:])
            b2_sb = cpool.tile([D, D], bf16)
            nc.sync.dma_start(out=b2_sb[:], in_=b2w[:])
            bp_sb = cpool.tile([D, 1], f32)
            nc.sync.dma_start(out=bp_sb[:], in_=bpc[:])
            io_sb = cpool.tile([BLK, TILE], bf16)
            nc.sync.dma_start(out=io_sb[:], in_=iota[:])
            xT_sb = cpool.tile([D, ROWS_PC], bf16)
            dl_lb = tpool.tile([BLK, NVIEWS], bf16)
            nc.scalar.dma_start(out=dl_lb[:], in_=dl[:])
            dd_lb = tpool.tile([BLK, NB], bf16)
            nc.scalar.dma_start(out=dd_lb[:], in_=dd[:])
            dl_sb = tpool.tile([BLK, NVIEWS], f32)
            nc.vector.tensor_copy(out=dl_sb[:], in_=dl_lb[:])
            dd_sb = tpool.tile([BLK, NB], f32)
            nc.vector.tensor_copy(out=dd_sb[:], in_=dd_lb[:])
            if int(grp_nb[0]) * 8 > sl0:
                nc.sync.dma_start(out=ix_sb[:, sl0:int(grp_nb[0]) * 8],
                                  in_=ix16[:, sl0:int(grp_nb[0]) * 8])
            for q in range(1, len(grp_slots)):
                a, b = int(grp_off[q]) * 8, (int(grp_off[q]) + int(grp_nb[q])) * 8
                nc.sync.dma_start(out=ix_sb[:, a:b], in_=ix16[:, a:b])
            nc.sync.dma_start(out=xT_sb[:], in_=xT[:])
            xsT_sb = cpool.tile([D, ROWS_PC], bf16)
            nc.sync.dma_start(out=xsT_sb[:], in_=xsT[:])

            qrr = [0]
            vcnt = [0]
            for q, sl in enumerate(grp_slots):
                gb0 = int(grp_off[q])
                gnb = int(grp_nb[q])
                g_sb = gpool.tile([BLK, gnb * D], bf16, tag="g")
                if "gather" not in ablate:
                    for (is_hi, b0, nb) in callplan[q]:
                        tbl = x_hi if is_hi else x_lo
                        nidx = nb * BLK
                        nc.gpsimd.dma_gather(
                            out_ap=g_sb[:, b0 * D:(b0 + nb) * D].rearrange(
                                "p (n e) -> p n e", e=D),
                            in_ap=tbl[:],
                            idxs_ap=ix_sb[:, (gb0 + b0) * 8:(gb0 + b0 + nb) * 8],
                            num_idxs=nidx,
                            num_idxs_reg=nidx,
                            elem_size=D,
                            queue_num=qrr[0] % N_SWDGE_QUEUES,
                            single_packet=False,
                        )
                        qrr[0] += 1
                og_sb = ogpool.tile([D, len(sl) * TILE], bf16, tag="og")
                zparts = {}
                for h in (0, 1):
                    for i in sl:
                        views = slot_views[i][h]
                        if not views or "segmm" in ablate:
                            continue
                        nv = len(views)
                        z_ps = zpool.tile([D, TILE], f32)
                        for jj, (gb, vcol) in enumerate(views):
                            lb = gb - gb0
                            d0, w = int(d0_v[vcol]), int(d1_v[vcol] - d0_v[vcol])
                            if "onehot" not in ablate:
                                oh = ohpool.tile([BLK, TILE], bf16, tag="oh")
                                nc.vector.tensor_scalar(
                                    out=oh[:, :w], in0=io_sb[:, :w],
                                    scalar1=dl_sb[:, vcol:vcol + 1],
                                    scalar2=dd_sb[:, gb:gb + 1],
                                    op0=eq, op1=mul,
                                )
                                rhs = oh[:, :w]
                            else:
                                rhs = io_sb[:, :w]
                            nc.tensor.matmul(
                                out=z_ps[:, d0:d0 + w],
                                lhsT=g_sb[:, lb * D:(lb + 1) * D],
                                rhs=rhs,
                                start=(jj == 0), stop=(jj == nv - 1),
                            )
                        z_sb = zsbpool.tile([D, TILE], bf16, tag="z")
                        nc.scalar.copy(out=z_sb[:], in_=z_ps[:])
                        zparts.setdefault(i, []).append(z_sb)
                if "epilogue" not in ablate:
                    for s, i in enumerate(sl):
                        o_ps = opool.tile([D, TILE], f32)
                        parts = zparts.get(i, [])
                        for z_sb in parts:
                            nc.tensor.matmul(out=o_ps[:], lhsT=c_sb[:],
                                             rhs=z_sb[:],
                                             start=(z_sb is parts[0]),
                                             stop=False)
                        nc.tensor.matmul(out=o_ps[:], lhsT=c_sb[:],
                                         rhs=xsT_sb[:, i * TILE:(i + 1) * TILE],
                                         start=(not parts), stop=False)
                        nc.tensor.matmul(out=o_ps[:], lhsT=b2_sb[:],
                                         rhs=xT_sb[:, i * TILE:(i + 1) * TILE],
                                         start=False, stop=True)
                        nc.scalar.activation(
                            out=og_sb[:, s * TILE:(s + 1) * TILE], in_=o_ps[:],
                            func=ident, bias=bp_sb[:, 0:1])
                if "epilogue" not in ablate:
                    col0 = sl[0] * TILE
                    eng_w = (nc.scalar if q == len(grp_slots) - 1 else nc.sync)
                    eng_w.dma_start(
                        out=out[:, col0:col0 + len(sl) * TILE], in_=og_sb[:])
    nc.compile()
    return nc


def _meta_key(meta):
    return (
        tuple(int(v) for v in meta["grp_nb"]),
        tuple(len(v) for v in meta["slot_views"]),
        int(meta["NVIEWS"]),
    )


def _get_nc(meta):
    key = _meta_key(meta)
    if key not in _NC_CACHE:
        _NC_CACHE[key] = _build_nc(meta)
    return _NC_CACHE[key]


def kernel(x, edge_index, Wc, bc, W0, Wt):
    global _LAST_RESULTS
    from concourse.bass_utils import run_bass_kernel_spmd

    x = np.asarray(x)
    n = x.shape[0]
    in_maps, meta = _host_prep(x, edge_index, Wc, bc, W0, Wt)
    nc = _get_nc(meta)
    res = run_bass_kernel_spmd(nc, in_maps, list(range(N_CORES)))
    _LAST_RESULTS = res
    out_full = np.empty((N_PAD, D), np.float32)
    for k in range(N_CORES):
        out_full[meta["node_ids"][k]] = np.asarray(
            res.results[k]["out"]).astype(np.float32).T
    return out_full[:n].astype(np.float32)
